# revision 9
# baseline (speedup 1.0000x reference)
"""ExampleTiedDropout (training path, first pass) on 8 Trainium2 cores.

out[b, c, h, w] = X[b, c, h, w] * mask[b, c]
  mask[b, :k]  = 1                      (k = int(0.2*C) fixed channels)
  mask[b, k:]  = bernoulli(p_mem=0.1)   seeded per example id idx[b]

Strategy: pure data parallel over the batch dim (8 examples per core).
The mask depends only on idx (64 ints) — it is computed host-side with
the exact same jax ops the reference uses (same backend -> bit-identical)
and shipped to the device as a tiny [128, n_tiles] tensor.  The device
kernel is a memory-bound broadcast multiply over X.
"""

import numpy as np

N_CORES = 8
P_FIXED, P_MEM, BASE_SEED = 0.2, 0.1, 1234

# The most recent BassKernelResults (exec_time_ns etc.) for test harnesses.
LAST_RESULT = None


def _ensure_ntff_hook():
    """Make BASS_TRACE=1 usable on images whose antenv lacks axon_hooks.

    concourse.bass_utils reads antenv.axon_hooks.get_axon_ntff_profile_hook()
    when tracing under axon; some images ship an antenv without that
    submodule, which turns trace=True into a hard ModuleNotFoundError.
    Register a small shim (and the ctypes-based hook from trn_agent_boot,
    when available) so tracing either works or degrades to a warning.
    """
    import importlib.util
    import sys
    import types

    try:
        if importlib.util.find_spec("antenv.axon_hooks") is not None:
            return
    except Exception:
        return
    mod = types.ModuleType("antenv.axon_hooks")
    mod._hook = None

    def set_axon_ntff_profile_hook(h):
        mod._hook = h

    def get_axon_ntff_profile_hook():
        return mod._hook

    mod.set_axon_ntff_profile_hook = set_axon_ntff_profile_hook
    mod.get_axon_ntff_profile_hook = get_axon_ntff_profile_hook
    sys.modules["antenv.axon_hooks"] = mod
    try:
        from trn_agent_boot.trn_boot import _ntff_profile_via_ctypes

        hook = _ntff_profile_via_ctypes("/opt/axon/libaxon_pjrt.so")
        if hook is not None:
            mod._hook = hook
    except Exception:
        pass


def _compute_mask(idx, C):
    """Per-example keep mask [B, C] float32; mirrors the reference's sampling."""
    import jax
    import jax.numpy as jnp

    k_fixed = int(P_FIXED * C)
    n_drop = C - k_fixed

    def _example_mask(example_id):
        key = jax.random.fold_in(jax.random.key(BASE_SEED), example_id)
        bern = (jax.random.uniform(key, (n_drop,)) < P_MEM).astype(jnp.float32)
        return jnp.concatenate([jnp.ones((k_fixed,), dtype=jnp.float32), bern])

    return np.asarray(jax.vmap(_example_mask)(jnp.asarray(idx)))


def _build_program(R, S, K=6):
    """One SPMD program: y[r, s] = x[r, s] * m[r%128, r//128] over R rows.

    Raw bass, three engine queues:
      SP  (sync):   mask DMA + tile loads     (inc s_in by 16 each)
      DVE (vector): per-tile broadcast multiply (inc s_mul by 1)
      ACT (scalar): tile stores                (inc s_out by 16 each)
    K SBUF slots, each instruction carries at most one semaphore wait.
    """
    from contextlib import ExitStack

    import concourse.bass as bass
    import concourse.mybir as mybir

    f32 = mybir.dt.float32
    P = 128
    n_tiles = R // P

    nc = bass.Bass()
    x = nc.declare_dram_parameter("x", [R, S], f32, isOutput=False)
    m = nc.declare_dram_parameter("m", [P, n_tiles], f32, isOutput=False)
    y = nc.declare_dram_parameter("y", [R, S], f32, isOutput=True)

    x_t = x.rearrange("(n p) s -> n p s", p=P)
    y_t = y.rearrange("(n p) s -> n p s", p=P)

    with ExitStack() as ctx:
        mt = ctx.enter_context(nc.sbuf_tensor([P, n_tiles], f32))
        tbuf = ctx.enter_context(nc.sbuf_tensor([P, K * S], f32))
        s_mask = ctx.enter_context(nc.semaphore("s_mask"))
        s_mul = ctx.enter_context(nc.semaphore("s_mul"))
        # Per-slot DMA semaphores: the dependency chain guarantees at most
        # one outstanding DMA per semaphore, so cumulative counts can't be
        # satisfied early by a *later* DMA completing out of order.
        s_in = [ctx.enter_context(nc.semaphore(f"s_in{k}")) for k in range(K)]
        s_out = [ctx.enter_context(nc.semaphore(f"s_out{k}")) for k in range(K)]
        block = ctx.enter_context(nc.Block())

        @block.sync
        def _(sync):
            sync.dma_start(out=mt[:, :], in_=m[:, :]).then_inc(s_mask, 16)
            for i in range(n_tiles):
                k = i % K
                if i >= K:
                    # store of the tile K iterations ago (same slot) done
                    sync.wait_ge(s_out[k], 16 * (i // K))
                sync.dma_start(
                    out=tbuf[:, k * S : k * S + S], in_=x_t[i]
                ).then_inc(s_in[k], 16)

        @block.vector
        def _(vector):
            vector.wait_ge(s_mask, 16)
            for i in range(n_tiles):
                k = i % K
                vector.wait_ge(s_in[k], 16 * (i // K + 1))
                vector.tensor_tensor(
                    out=tbuf[:, k * S : k * S + S],
                    in0=tbuf[:, k * S : k * S + S],
                    in1=mt[:, i : i + 1].to_broadcast([P, S]),
                    op=mybir.AluOpType.mult,
                ).then_inc(s_mul, 1)

        @block.scalar
        def _(scalar):
            for i in range(n_tiles):
                k = i % K
                scalar.wait_ge(s_mul, i + 1)
                scalar.dma_start(
                    out=y_t[i], in_=tbuf[:, k * S : k * S + S]
                ).then_inc(s_out[k], 16)
            for k in range(K):
                n_k = len(range(k, n_tiles, K))
                scalar.wait_ge(s_out[k], 16 * n_k)

    return nc


def _run_spmd(nc, in_maps, core_ids):
    import os

    from concourse.bass_utils import run_bass_kernel_spmd

    try:
        return run_bass_kernel_spmd(nc, in_maps, core_ids)
    except Exception:
        # A broken tracing stack (e.g. BASS_TRACE=1 with no NTFF hook
        # backend) must not take correctness down with it.
        old = os.environ.get("BASS_NEVER_TRACE")
        os.environ["BASS_NEVER_TRACE"] = "1"
        try:
            return run_bass_kernel_spmd(nc, in_maps, core_ids)
        finally:
            if old is None:
                os.environ.pop("BASS_NEVER_TRACE", None)
            else:
                os.environ["BASS_NEVER_TRACE"] = old


def kernel(X, idx):
    global LAST_RESULT
    _ensure_ntff_hook()

    X = np.ascontiguousarray(np.asarray(X, dtype=np.float32))
    B, C, H, W = X.shape
    S = H * W
    mask = _compute_mask(idx, C)  # [B, C] float32, exactly 0.0/1.0

    per = B // N_CORES
    R = per * C  # rows per core; row = b_local*C + c
    P = 128
    n_tiles = R // P

    nc = _build_program(R, S)

    Xf = X.reshape(N_CORES, R, S)
    Mf = mask.astype(np.float32).reshape(N_CORES, R)
    in_maps = []
    for c in range(N_CORES):
        m_t = np.ascontiguousarray(Mf[c].reshape(n_tiles, P).T)  # [128, n_tiles]
        in_maps.append({"x": Xf[c], "m": m_t})

    LAST_RESULT = _run_spmd(nc, in_maps, list(range(N_CORES)))
    out = np.stack([LAST_RESULT.results[i]["y"] for i in range(N_CORES)])
    return out.reshape(B, C, H, W).astype(np.float32, copy=False)


# revision 12
# speedup vs baseline: 3.4288x; 3.4288x over previous
"""ExampleTiedDropout (training path, first pass) on 8 Trainium2 cores.

out[b, c, h, w] = X[b, c, h, w] * mask[b, c]
  mask[b, :k]  = 1                      (k = int(0.2*C) fixed channels)
  mask[b, k:]  = bernoulli(p_mem=0.1)   seeded per example id idx[b]

Strategy: pure data parallel over the batch dim (8 examples per core).
The mask depends only on idx (64 ints) — it is computed host-side with
the exact same jax ops the reference uses (same backend -> bit-identical)
and shipped to the device as a tiny [128, n_tiles] tensor.  The device
kernel is a memory-bound broadcast multiply over X.
"""

import numpy as np

N_CORES = 8
P_FIXED, P_MEM, BASE_SEED = 0.2, 0.1, 1234

# The most recent BassKernelResults (exec_time_ns etc.) for test harnesses.
LAST_RESULT = None


def _ensure_ntff_hook():
    """Make BASS_TRACE=1 usable on images whose antenv lacks axon_hooks.

    concourse.bass_utils reads antenv.axon_hooks.get_axon_ntff_profile_hook()
    when tracing under axon; some images ship an antenv without that
    submodule, which turns trace=True into a hard ModuleNotFoundError.
    Register a small shim (and the ctypes-based hook from trn_agent_boot,
    when available) so tracing either works or degrades to a warning.
    """
    import importlib.util
    import sys
    import types

    try:
        if importlib.util.find_spec("antenv.axon_hooks") is not None:
            return
    except Exception:
        return
    mod = types.ModuleType("antenv.axon_hooks")
    mod._hook = None

    def set_axon_ntff_profile_hook(h):
        mod._hook = h

    def get_axon_ntff_profile_hook():
        return mod._hook

    mod.set_axon_ntff_profile_hook = set_axon_ntff_profile_hook
    mod.get_axon_ntff_profile_hook = get_axon_ntff_profile_hook
    sys.modules["antenv.axon_hooks"] = mod
    try:
        from trn_agent_boot.trn_boot import _ntff_profile_via_ctypes

        hook = _ntff_profile_via_ctypes("/opt/axon/libaxon_pjrt.so")
        if hook is not None:
            mod._hook = hook
    except Exception:
        pass


def _compute_mask(idx, C):
    """Per-example keep mask [B, C] float32; mirrors the reference's sampling."""
    import jax
    import jax.numpy as jnp

    k_fixed = int(P_FIXED * C)
    n_drop = C - k_fixed

    def _example_mask(example_id):
        key = jax.random.fold_in(jax.random.key(BASE_SEED), example_id)
        bern = (jax.random.uniform(key, (n_drop,)) < P_MEM).astype(jnp.float32)
        return jnp.concatenate([jnp.ones((k_fixed,), dtype=jnp.float32), bern])

    return np.asarray(jax.vmap(_example_mask)(jnp.asarray(idx)))


def _build_program(R, S, K=6):
    """One SPMD program: y[r, s] = x[r, s] * m[r%128, r//128] over R rows.

    Raw bass, three engine queues:
      SP  (sync):   mask DMA + tile loads     (inc s_in by 16 each)
      DVE (vector): per-tile broadcast multiply (inc s_mul by 1)
      ACT (scalar): tile stores                (inc s_out by 16 each)
    K SBUF slots, each instruction carries at most one semaphore wait.
    """
    from contextlib import ExitStack

    import concourse.bass as bass
    import concourse.mybir as mybir

    f32 = mybir.dt.float32
    P = 128
    n_tiles = R // P

    nc = bass.Bass()
    x = nc.declare_dram_parameter("x", [R, S], f32, isOutput=False)
    m = nc.declare_dram_parameter("m", [P, n_tiles], f32, isOutput=False)
    y = nc.declare_dram_parameter("y", [R, S], f32, isOutput=True)

    x_t = x.rearrange("(n p) s -> n p s", p=P)
    y_t = y.rearrange("(n p) s -> n p s", p=P)

    with ExitStack() as ctx:
        mt = ctx.enter_context(nc.sbuf_tensor([P, n_tiles], f32))
        tbuf = ctx.enter_context(nc.sbuf_tensor([P, K * S], f32))
        s_mask = ctx.enter_context(nc.semaphore("s_mask"))
        s_mul = ctx.enter_context(nc.semaphore("s_mul"))
        # Per-slot DMA semaphores: the dependency chain guarantees at most
        # one outstanding DMA per semaphore, so cumulative counts can't be
        # satisfied early by a *later* DMA completing out of order.
        s_in = [ctx.enter_context(nc.semaphore(f"s_in{k}")) for k in range(K)]
        s_out = [ctx.enter_context(nc.semaphore(f"s_out{k}")) for k in range(K)]
        block = ctx.enter_context(nc.Block())

        @block.sync
        def _(sync):
            sync.dma_start(out=mt[:, :], in_=m[:, :]).then_inc(s_mask, 16)
            for i in range(n_tiles):
                k = i % K
                if i >= K:
                    # store of the tile K iterations ago (same slot) done
                    sync.wait_ge(s_out[k], 16 * (i // K))
                sync.dma_start(
                    out=tbuf[:, k * S : k * S + S], in_=x_t[i]
                ).then_inc(s_in[k], 16)

        @block.vector
        def _(vector):
            vector.wait_ge(s_mask, 16)
            for i in range(n_tiles):
                k = i % K
                vector.wait_ge(s_in[k], 16 * (i // K + 1))
                vector.tensor_tensor(
                    out=tbuf[:, k * S : k * S + S],
                    in0=tbuf[:, k * S : k * S + S],
                    in1=mt[:, i : i + 1].to_broadcast([P, S]),
                    op=mybir.AluOpType.mult,
                ).then_inc(s_mul, 1)

        @block.scalar
        def _(scalar):
            for i in range(n_tiles):
                k = i % K
                scalar.wait_ge(s_mul, i + 1)
                scalar.dma_start(
                    out=y_t[i], in_=tbuf[:, k * S : k * S + S]
                ).then_inc(s_out[k], 16)
            for k in range(K):
                n_k = len(range(k, n_tiles, K))
                scalar.wait_ge(s_out[k], 16 * n_k)

    return nc


def _run_spmd(nc, in_maps, core_ids):
    import os

    from concourse.bass_utils import run_bass_kernel_spmd

    try:
        return run_bass_kernel_spmd(nc, in_maps, core_ids)
    except Exception:
        # A broken tracing stack (e.g. BASS_TRACE=1 with no NTFF hook
        # backend) must not take correctness down with it.
        old = os.environ.get("BASS_NEVER_TRACE")
        os.environ["BASS_NEVER_TRACE"] = "1"
        try:
            return run_bass_kernel_spmd(nc, in_maps, core_ids)
        finally:
            if old is None:
                os.environ.pop("BASS_NEVER_TRACE", None)
            else:
                os.environ["BASS_NEVER_TRACE"] = old


def _build_sparse_program(R, S, n_fix_blocks, fix_rows, C, G, n_slots=8):
    """Copy only the kept rows of x into y (y is pre-zeroed by the runtime).

    - Fixed channels [0, fix_rows) of each example: n_fix_blocks direct
      HBM->HBM block copies (static structure, same on every core).
    - Bernoulli-kept rows: G rounds of indirect gather (x -> SBUF) +
      indirect scatter (SBUF -> y), 128 row indices per round, fed by a
      per-core [128, G] int32 index tensor.  Padding entries use index R
      (out of bounds) and are silently skipped via bounds_check.
    """
    from contextlib import ExitStack

    import concourse.bass as bass
    import concourse.mybir as mybir

    f32 = mybir.dt.float32
    i32 = mybir.dt.int32
    P = 128

    nc = bass.Bass()
    x = nc.declare_dram_parameter("x", [R, S], f32, isOutput=False)
    g = nc.declare_dram_parameter("g", [P, max(G, 1)], i32, isOutput=False)
    y = nc.declare_dram_parameter("y", [R, S], f32, isOutput=True)

    n_slots = min(n_slots, max(G, 1))

    with ExitStack() as ctx:
        gt = ctx.enter_context(nc.sbuf_tensor([P, max(G, 1)], i32))
        dbuf = ctx.enter_context(nc.sbuf_tensor([P, n_slots * S], f32))
        s_fix = ctx.enter_context(nc.semaphore("s_fix"))
        s_gidx = ctx.enter_context(nc.semaphore("s_gidx"))
        s_ga = [ctx.enter_context(nc.semaphore(f"s_ga{k}")) for k in range(n_slots)]
        s_sc = [ctx.enter_context(nc.semaphore(f"s_sc{k}")) for k in range(n_slots)]
        block = ctx.enter_context(nc.Block())

        @block.sync
        def _(sync):
            for b in range(n_fix_blocks):
                r0 = b * C
                sync.dma_start(
                    out=y[r0 : r0 + fix_rows, :], in_=x[r0 : r0 + fix_rows, :]
                ).then_inc(s_fix, 16)
            sync.wait_ge(s_fix, 16 * n_fix_blocks)

        @block.gpsimd
        def _(gpsimd):
            if G == 0:
                return
            gpsimd.dma_start(out=gt[:, :], in_=g[:, :]).then_inc(s_gidx, 16)
            gpsimd.wait_ge(s_gidx, 16)

            def gather(j):
                k = j % n_slots
                gpsimd.indirect_dma_start(
                    out=dbuf[:, k * S : k * S + S],
                    out_offset=None,
                    in_=x[:, :],
                    in_offset=bass.IndirectOffsetOnAxis(ap=gt[:, j : j + 1], axis=0),
                    bounds_check=R - 1,
                    oob_is_err=False,
                ).then_inc(s_ga[k], 16)

            # first wave of gathers issues back-to-back so they overlap;
            # each scatter is gated on its own gather, and a reused slot's
            # next gather is gated on that scatter having drained
            for j in range(min(G, n_slots)):
                gather(j)
            for j in range(G):
                k = j % n_slots
                gpsimd.wait_ge(s_ga[k], 16 * (j // n_slots + 1))
                gpsimd.indirect_dma_start(
                    out=y[:, :],
                    out_offset=bass.IndirectOffsetOnAxis(ap=gt[:, j : j + 1], axis=0),
                    in_=dbuf[:, k * S : k * S + S],
                    in_offset=None,
                    bounds_check=R - 1,
                    oob_is_err=False,
                ).then_inc(s_sc[k], 16)
                if j + n_slots < G:
                    gpsimd.wait_ge(s_sc[k], 16 * (j // n_slots + 1))
                    gather(j + n_slots)
            for k in range(n_slots):
                n_k = len(range(k, G, n_slots))
                if n_k:
                    gpsimd.wait_ge(s_sc[k], 16 * n_k)

    return nc


def kernel(X, idx):
    global LAST_RESULT
    _ensure_ntff_hook()

    import os

    X = np.ascontiguousarray(np.asarray(X, dtype=np.float32))
    B, C, H, W = X.shape
    S = H * W
    mask = _compute_mask(idx, C)  # [B, C] float32, exactly 0.0/1.0

    per = B // N_CORES
    R = per * C  # rows per core; row = b_local*C + c
    P = 128
    n_tiles = R // P

    Xf = X.reshape(N_CORES, R, S)
    Mf = (mask.astype(np.float32).reshape(N_CORES, R) != 0.0)

    dense = os.environ.get("TIED_DROPOUT_DENSE", "0") == "1"
    if dense:
        nc = _build_program(R, S)
        in_maps = []
        for c in range(N_CORES):
            m_t = np.ascontiguousarray(
                Mf[c].astype(np.float32).reshape(n_tiles, P).T
            )  # [128, n_tiles]
            in_maps.append({"x": Xf[c], "m": m_t})
    else:
        k_fixed = int(P_FIXED * C)
        # data-dependent scattered rows: the bernoulli-kept channels
        kept = [np.nonzero(Mf[c])[0].astype(np.int32) for c in range(N_CORES)]
        bern = [k[(k % C) >= k_fixed] for k in kept]
        G = (max(len(b) for b in bern) + P - 1) // P  # index tiles per core
        in_maps = []
        for c in range(N_CORES):
            gidx = np.full(max(G, 1) * P, R, dtype=np.int32)  # R = skipped pad
            gidx[: len(bern[c])] = bern[c]
            g_t = np.ascontiguousarray(gidx.reshape(max(G, 1), P).T)  # [128, G]
            in_maps.append({"x": Xf[c], "g": g_t})
        nc = _build_sparse_program(R, S, per, k_fixed, C, G)

    LAST_RESULT = _run_spmd(nc, in_maps, list(range(N_CORES)))
    out = np.stack([LAST_RESULT.results[i]["y"] for i in range(N_CORES)])
    return out.reshape(B, C, H, W).astype(np.float32, copy=False)


# revision 13
# speedup vs baseline: 3.4792x; 1.0147x over previous
"""ExampleTiedDropout (training path, first pass) on 8 Trainium2 cores.

out[b, c, h, w] = X[b, c, h, w] * mask[b, c]
  mask[b, :k]  = 1                      (k = int(0.2*C) fixed channels)
  mask[b, k:]  = bernoulli(p_mem=0.1)   seeded per example id idx[b]

Strategy: pure data parallel over the batch dim (8 examples per core).
The mask depends only on idx (64 ints) — it is computed host-side with
the exact same jax ops the reference uses (same backend -> bit-identical)
and shipped to the device as a tiny [128, n_tiles] tensor.  The device
kernel is a memory-bound broadcast multiply over X.
"""

import numpy as np

N_CORES = 8
P_FIXED, P_MEM, BASE_SEED = 0.2, 0.1, 1234

# The most recent BassKernelResults (exec_time_ns etc.) for test harnesses.
LAST_RESULT = None


def _ensure_ntff_hook():
    """Make BASS_TRACE=1 usable on images whose antenv lacks axon_hooks.

    concourse.bass_utils reads antenv.axon_hooks.get_axon_ntff_profile_hook()
    when tracing under axon; some images ship an antenv without that
    submodule, which turns trace=True into a hard ModuleNotFoundError.
    Register a small shim (and the ctypes-based hook from trn_agent_boot,
    when available) so tracing either works or degrades to a warning.
    """
    import importlib.util
    import sys
    import types

    try:
        if importlib.util.find_spec("antenv.axon_hooks") is not None:
            return
    except Exception:
        return
    mod = types.ModuleType("antenv.axon_hooks")
    mod._hook = None

    def set_axon_ntff_profile_hook(h):
        mod._hook = h

    def get_axon_ntff_profile_hook():
        return mod._hook

    mod.set_axon_ntff_profile_hook = set_axon_ntff_profile_hook
    mod.get_axon_ntff_profile_hook = get_axon_ntff_profile_hook
    sys.modules["antenv.axon_hooks"] = mod
    try:
        from trn_agent_boot.trn_boot import _ntff_profile_via_ctypes

        hook = _ntff_profile_via_ctypes("/opt/axon/libaxon_pjrt.so")
        if hook is not None:
            mod._hook = hook
    except Exception:
        pass


def _compute_mask(idx, C):
    """Per-example keep mask [B, C] float32; mirrors the reference's sampling."""
    import jax
    import jax.numpy as jnp

    k_fixed = int(P_FIXED * C)
    n_drop = C - k_fixed

    def _example_mask(example_id):
        key = jax.random.fold_in(jax.random.key(BASE_SEED), example_id)
        bern = (jax.random.uniform(key, (n_drop,)) < P_MEM).astype(jnp.float32)
        return jnp.concatenate([jnp.ones((k_fixed,), dtype=jnp.float32), bern])

    return np.asarray(jax.vmap(_example_mask)(jnp.asarray(idx)))


def _build_program(R, S, K=6):
    """One SPMD program: y[r, s] = x[r, s] * m[r%128, r//128] over R rows.

    Raw bass, three engine queues:
      SP  (sync):   mask DMA + tile loads     (inc s_in by 16 each)
      DVE (vector): per-tile broadcast multiply (inc s_mul by 1)
      ACT (scalar): tile stores                (inc s_out by 16 each)
    K SBUF slots, each instruction carries at most one semaphore wait.
    """
    from contextlib import ExitStack

    import concourse.bass as bass
    import concourse.mybir as mybir

    f32 = mybir.dt.float32
    P = 128
    n_tiles = R // P

    nc = bass.Bass()
    x = nc.declare_dram_parameter("x", [R, S], f32, isOutput=False)
    m = nc.declare_dram_parameter("m", [P, n_tiles], f32, isOutput=False)
    y = nc.declare_dram_parameter("y", [R, S], f32, isOutput=True)

    x_t = x.rearrange("(n p) s -> n p s", p=P)
    y_t = y.rearrange("(n p) s -> n p s", p=P)

    with ExitStack() as ctx:
        mt = ctx.enter_context(nc.sbuf_tensor([P, n_tiles], f32))
        tbuf = ctx.enter_context(nc.sbuf_tensor([P, K * S], f32))
        s_mask = ctx.enter_context(nc.semaphore("s_mask"))
        s_mul = ctx.enter_context(nc.semaphore("s_mul"))
        # Per-slot DMA semaphores: the dependency chain guarantees at most
        # one outstanding DMA per semaphore, so cumulative counts can't be
        # satisfied early by a *later* DMA completing out of order.
        s_in = [ctx.enter_context(nc.semaphore(f"s_in{k}")) for k in range(K)]
        s_out = [ctx.enter_context(nc.semaphore(f"s_out{k}")) for k in range(K)]
        block = ctx.enter_context(nc.Block())

        @block.sync
        def _(sync):
            sync.dma_start(out=mt[:, :], in_=m[:, :]).then_inc(s_mask, 16)
            for i in range(n_tiles):
                k = i % K
                if i >= K:
                    # store of the tile K iterations ago (same slot) done
                    sync.wait_ge(s_out[k], 16 * (i // K))
                sync.dma_start(
                    out=tbuf[:, k * S : k * S + S], in_=x_t[i]
                ).then_inc(s_in[k], 16)

        @block.vector
        def _(vector):
            vector.wait_ge(s_mask, 16)
            for i in range(n_tiles):
                k = i % K
                vector.wait_ge(s_in[k], 16 * (i // K + 1))
                vector.tensor_tensor(
                    out=tbuf[:, k * S : k * S + S],
                    in0=tbuf[:, k * S : k * S + S],
                    in1=mt[:, i : i + 1].to_broadcast([P, S]),
                    op=mybir.AluOpType.mult,
                ).then_inc(s_mul, 1)

        @block.scalar
        def _(scalar):
            for i in range(n_tiles):
                k = i % K
                scalar.wait_ge(s_mul, i + 1)
                scalar.dma_start(
                    out=y_t[i], in_=tbuf[:, k * S : k * S + S]
                ).then_inc(s_out[k], 16)
            for k in range(K):
                n_k = len(range(k, n_tiles, K))
                scalar.wait_ge(s_out[k], 16 * n_k)

    return nc


def _run_spmd(nc, in_maps, core_ids):
    import os

    from concourse.bass_utils import run_bass_kernel_spmd

    try:
        return run_bass_kernel_spmd(nc, in_maps, core_ids)
    except Exception:
        # A broken tracing stack (e.g. BASS_TRACE=1 with no NTFF hook
        # backend) must not take correctness down with it.
        old = os.environ.get("BASS_NEVER_TRACE")
        os.environ["BASS_NEVER_TRACE"] = "1"
        try:
            return run_bass_kernel_spmd(nc, in_maps, core_ids)
        finally:
            if old is None:
                os.environ.pop("BASS_NEVER_TRACE", None)
            else:
                os.environ["BASS_NEVER_TRACE"] = old


def _build_sparse_program(R, S, n_fix_blocks, fix_rows, C, G, n_slots=8):
    """Copy only the kept rows of x into y (y is pre-zeroed by the runtime).

    - Fixed channels [0, fix_rows) of each example: n_fix_blocks direct
      HBM->HBM block copies (static structure, same on every core).
    - Bernoulli-kept rows: G rounds of indirect gather (x -> SBUF) +
      indirect scatter (SBUF -> y), 128 row indices per round, fed by a
      per-core [128, G] int32 index tensor.  Padding entries use index R
      (out of bounds) and are silently skipped via bounds_check.
    """
    from contextlib import ExitStack

    import concourse.bass as bass
    import concourse.mybir as mybir

    f32 = mybir.dt.float32
    i32 = mybir.dt.int32
    P = 128

    nc = bass.Bass()
    x = nc.declare_dram_parameter("x", [R, S], f32, isOutput=False)
    g = nc.declare_dram_parameter("g", [P, max(G, 1)], i32, isOutput=False)
    y = nc.declare_dram_parameter("y", [R, S], f32, isOutput=True)

    n_slots = min(n_slots, max(G, 1))

    with ExitStack() as ctx:
        gt = ctx.enter_context(nc.sbuf_tensor([P, max(G, 1)], i32))
        dbuf = ctx.enter_context(nc.sbuf_tensor([P, n_slots * S], f32))
        s_fix = ctx.enter_context(nc.semaphore("s_fix"))
        s_gidx = ctx.enter_context(nc.semaphore("s_gidx"))
        s_ga = [ctx.enter_context(nc.semaphore(f"s_ga{k}")) for k in range(n_slots)]
        s_sc = [ctx.enter_context(nc.semaphore(f"s_sc{k}")) for k in range(n_slots)]
        block = ctx.enter_context(nc.Block())

        @block.sync
        def _(sync):
            # index table first: it is tiny and the gpsimd gathers are
            # gated on it, so don't let it queue behind the block copies
            if G > 0:
                sync.dma_start(out=gt[:, :], in_=g[:, :]).then_inc(s_gidx, 16)
            for b in range(0, n_fix_blocks, 2):
                r0 = b * C
                sync.dma_start(
                    out=y[r0 : r0 + fix_rows, :], in_=x[r0 : r0 + fix_rows, :]
                ).then_inc(s_fix, 16)
            sync.wait_ge(s_fix, 16 * n_fix_blocks)

        @block.scalar
        def _(scalar):
            for b in range(1, n_fix_blocks, 2):
                r0 = b * C
                scalar.dma_start(
                    out=y[r0 : r0 + fix_rows, :], in_=x[r0 : r0 + fix_rows, :]
                ).then_inc(s_fix, 16)

        @block.gpsimd
        def _(gpsimd):
            if G == 0:
                return
            gpsimd.wait_ge(s_gidx, 16)

            def gather(j):
                k = j % n_slots
                gpsimd.indirect_dma_start(
                    out=dbuf[:, k * S : k * S + S],
                    out_offset=None,
                    in_=x[:, :],
                    in_offset=bass.IndirectOffsetOnAxis(ap=gt[:, j : j + 1], axis=0),
                    bounds_check=R - 1,
                    oob_is_err=False,
                ).then_inc(s_ga[k], 16)

            # first wave of gathers issues back-to-back so they overlap;
            # each scatter is gated on its own gather, and a reused slot's
            # next gather is gated on that scatter having drained
            for j in range(min(G, n_slots)):
                gather(j)
            for j in range(G):
                k = j % n_slots
                gpsimd.wait_ge(s_ga[k], 16 * (j // n_slots + 1))
                gpsimd.indirect_dma_start(
                    out=y[:, :],
                    out_offset=bass.IndirectOffsetOnAxis(ap=gt[:, j : j + 1], axis=0),
                    in_=dbuf[:, k * S : k * S + S],
                    in_offset=None,
                    bounds_check=R - 1,
                    oob_is_err=False,
                ).then_inc(s_sc[k], 16)
                if j + n_slots < G:
                    gpsimd.wait_ge(s_sc[k], 16 * (j // n_slots + 1))
                    gather(j + n_slots)
            for k in range(n_slots):
                n_k = len(range(k, G, n_slots))
                if n_k:
                    gpsimd.wait_ge(s_sc[k], 16 * n_k)

    return nc


def kernel(X, idx):
    global LAST_RESULT
    _ensure_ntff_hook()

    import os

    X = np.ascontiguousarray(np.asarray(X, dtype=np.float32))
    B, C, H, W = X.shape
    S = H * W
    mask = _compute_mask(idx, C)  # [B, C] float32, exactly 0.0/1.0

    per = B // N_CORES
    R = per * C  # rows per core; row = b_local*C + c
    P = 128
    n_tiles = R // P

    Xf = X.reshape(N_CORES, R, S)
    Mf = (mask.astype(np.float32).reshape(N_CORES, R) != 0.0)

    dense = os.environ.get("TIED_DROPOUT_DENSE", "0") == "1"
    if dense:
        nc = _build_program(R, S)
        in_maps = []
        for c in range(N_CORES):
            m_t = np.ascontiguousarray(
                Mf[c].astype(np.float32).reshape(n_tiles, P).T
            )  # [128, n_tiles]
            in_maps.append({"x": Xf[c], "m": m_t})
    else:
        k_fixed = int(P_FIXED * C)
        # data-dependent scattered rows: the bernoulli-kept channels
        kept = [np.nonzero(Mf[c])[0].astype(np.int32) for c in range(N_CORES)]
        bern = [k[(k % C) >= k_fixed] for k in kept]
        G = (max(len(b) for b in bern) + P - 1) // P  # index tiles per core
        in_maps = []
        for c in range(N_CORES):
            gidx = np.full(max(G, 1) * P, R, dtype=np.int32)  # R = skipped pad
            gidx[: len(bern[c])] = bern[c]
            g_t = np.ascontiguousarray(gidx.reshape(max(G, 1), P).T)  # [128, G]
            in_maps.append({"x": Xf[c], "g": g_t})
        nc = _build_sparse_program(R, S, per, k_fixed, C, G)

    LAST_RESULT = _run_spmd(nc, in_maps, list(range(N_CORES)))
    out = np.stack([LAST_RESULT.results[i]["y"] for i in range(N_CORES)])
    return out.reshape(B, C, H, W).astype(np.float32, copy=False)
